# revision 6
# baseline (speedup 1.0000x reference)
"""MoE routed dynamics kernel for Trainium2 (8 NeuronCores, expert-parallel).

Problem: for each row b of a [B, D+A] input, route through one of P=8
two-layer MLPs selected by policy_indices[b]:
    h = relu(x @ W1[p] + b1[p]);  y = h @ W2[p] + b2[p]

Sharding: expert-parallel. Core p owns expert p's weights and processes
the rows routed to expert p (host-side gather by expert, padded to a
common capacity C; inverse scatter at unshard).

Design (v5; all constants from trace analysis on this silicon):
- Matmul dtype is float32r (N=512 pitch ~231ns vs bf16 ~259ns; fp32r
  needs N>=256 for 1 cycle/col), EXCEPT chunk0's L1 which runs bf16
  directly: inputs are already bf16-rounded in DRAM so bf16 MACs are
  numerically identical, and it removes the x0 upcast from the startup
  critical path (first-transfer DMA latency is ~3.7us after doorbell,
  so the first real matmul lands ~11.3us; warmups ramp the HAM clock
  gate meanwhile).
- Per-DGE-ring DMA bandwidth is far below the ~358GB/s bus, so the load
  stream is spread across three rings in consumption order, with W1
  m-blocks alternating scalar/gpsimd so their arrival cadence beats
  chunk0's ~0.6us/block consumption. x2+ and W2 are released by a
  gpsimd read gated on chunk0's first relu (in-order-queue flow
  control) so they never compete with startup-critical loads.
- Everything stages bf16 (half bus bytes): x, W1, W2. All upcasts run
  on DVE only (gpsimd CAST is ~5x slower, and concurrent DVE+gpsimd
  casts throttle each other ~19x), emitted in cast-deadline order.
- W1 is m-block-major in DRAM; odd m-blocks' k4 weights are relocated
  to partitions 64..127 with x's k4 rows 64..127 duplicating rows
  0..63 (layout also supports a packed row-tiled k4, which measured
  net-negative due to the sub-row LDWEIGHTS drain bubble).
- L1 keeps all h tiles resident in SBUF; L2 runs chunks in reverse so
  the tail drains the small lead-in chunk, whose d0..d2 groups are
  flushed early; output stored bf16, upcast on host.
"""

import math

import numpy as np
import ml_dtypes

_B = 16384
_P = 8
_D = 512
_A = 64
_H = 1024
_DA = _D + _A   # 576
_KC = 5         # K chunks over DA padded to 5*128=640
_N_CORES = 8
_MH = _H // 128  # 8 L1 output groups
_MD = _D // 128  # 4 L2 output groups

_WARMUP_N = 256
_WARMUPS = 12

_kernel_cache: dict = {}


def _chunks(C: int):
    """Column chunking: 256-wide lead-in (small first x transfer, so the
    PE starts sooner), then 512-wide steady chunks and a >=256 tail
    (fp32r needs N>=256 for full rate)."""
    assert C >= 256, C
    if C <= 512:
        return [C]
    out = [256]
    rem = C - 256
    while rem >= 1024:
        out.append(512)
        rem -= 512
    if rem <= 512:
        out.append(rem)
    elif rem - 512 >= 256:
        out += [512, rem - 512]
    else:
        out += [rem - 256, 256]
    return out


def _build_bass(C: int):
    import concourse.bacc as bacc
    import concourse.mybir as mybir
    from concourse.tile import TileContext

    fp32 = mybir.dt.float32
    f32r = mybir.dt.float32r
    bf16 = mybir.dt.bfloat16
    act = mybir.ActivationFunctionType

    widths = _chunks(C)
    offsets = [sum(widths[:i]) for i in range(len(widths))]
    mh, md = _MH, _MD

    nc = bacc.Bacc()
    xd = nc.declare_dram_parameter("xq", [128, _KC * C], bf16, isOutput=False)
    w1d = nc.declare_dram_parameter("w1q", [128, mh, _KC * 128], bf16, isOutput=False)
    w2d = nc.declare_dram_parameter("w2q", [128, mh, _D], bf16, isOutput=False)
    bd = nc.declare_dram_parameter("bq", [128, mh + md], fp32, isOutput=False)
    od = nc.declare_dram_parameter("oq", [128, md, C], bf16, isOutput=True)

    with TileContext(nc) as tc:
        with (
            tc.tile_pool(name="wpool", bufs=1) as wpool,
            tc.tile_pool(name="xpool", bufs=1) as xpool,
            tc.tile_pool(name="hpool", bufs=1) as hpool,
            tc.tile_pool(name="ypool", bufs=2) as ypool,
            tc.tile_pool(name="psum", bufs=8, space="PSUM") as psp,
        ):
            w1b_sb = wpool.tile([128, mh, _KC, 128], bf16, tag="w1b")
            w1_sb = wpool.tile([128, mh, _KC, 128], f32r, tag="w1")
            w2b_sb = wpool.tile([128, mh, _D], bf16, tag="w2b")
            w2_sb = wpool.tile([128, mh, _D], f32r, tag="w2")
            b_sb = wpool.tile([128, mh + md], fp32, tag="b")
            warm_sb = wpool.tile([128, 128 + _WARMUP_N], bf16, tag="warm")
            scratch = wpool.tile([128, 1], f32r, tag="scratch")

            xb_sb = [
                xpool.tile([128, _KC, nl], bf16, tag=f"xb{ci}", name=f"xb{ci}")
                for ci, nl in enumerate(widths)
            ]
            x_sb = [None] + [
                xpool.tile([128, _KC, nl], f32r, tag=f"x{ci}", name=f"x{ci}")
                for ci, nl in enumerate(widths[1:], start=1)
            ]

            nc.vector.memset(warm_sb[:, :], 0.0)

            # Per-transfer DMA latency is ~2-2.5us once several rings
            # compete, so W1 ships as four 2-m-block transfers (328KB)
            # alternating scalar/gpsimd — cadence ~2 blocks / 1.4us vs
            # chunk0's ~0.6us/block consumption. x0 (startup-critical),
            # biases, and x1 ride sync.
            nc.sync.dma_start(out=xb_sb[0][:, :, :], in_=xd[:, 0 : _KC * widths[0]])
            nc.sync.dma_start(out=b_sb[:, :], in_=bd[:, :])
            nc.sync.dma_start(
                out=xb_sb[1][:, :, :],
                in_=xd[:, _KC * offsets[1] : _KC * (offsets[1] + widths[1])],
            )
            nc.scalar.dma_start(out=w1b_sb[:, 0:2, :, :], in_=w1d[:, 0:2, :])
            nc.gpsimd.dma_start(out=w1b_sb[:, 2:4, :, :], in_=w1d[:, 2:4, :])
            nc.scalar.dma_start(out=w1b_sb[:, 4:6, :, :], in_=w1d[:, 4:6, :])
            nc.gpsimd.dma_start(out=w1b_sb[:, 6:8, :, :], in_=w1d[:, 6:8, :])

            # All upcasts on DVE (deadline order; the f32r W1 blocks are
            # first needed by chunk1, x1 likewise).
            for m in range(0, 2):
                nc.vector.tensor_copy(w1_sb[:, m, :, :], w1b_sb[:, m, :, :])
            nc.vector.tensor_copy(x_sb[1][:, :, :], xb_sb[1][:, :, :])
            for m in range(2, mh):
                nc.vector.tensor_copy(w1_sb[:, m, :, :], w1b_sb[:, m, :, :])

            # PE warmup: the clock gate holds the PE at reduced rate for
            # ~3.4us of sustained activity; burn the x0 DMA wait.
            for _ in range(_WARMUPS):
                wp = psp.tile([128, _WARMUP_N], fp32, tag="ps", name="warmps")
                nc.tensor.matmul(
                    wp[:, :], warm_sb[:, 0:128], warm_sb[:, 128 : 128 + _WARMUP_N],
                    start=True, stop=True,
                )

            # Phase 1: L1 (h = relu(W1.T x + b1)). chunk0 consumes the
            # bf16 tiles directly; later chunks use the f32r upcasts.
            # All h tiles stay resident in SBUF.
            h_sb: dict = {}
            for ci, nl in enumerate(widths):
                if ci == 0:
                    w_of = lambda m, k: w1b_sb[:, m, k, :]
                    xc = xb_sb[0]
                else:
                    w_of = lambda m, k: w1_sb[:, m, k, :]
                    xc = x_sb[ci]
                for m in range(mh):
                    ps = psp.tile([128, nl], fp32, tag="ps", name=f"ps1_{ci}_{m}")
                    for k in range(_KC):
                        nc.tensor.matmul(
                            ps[:, :],
                            w_of(m, k),
                            xc[:, k, :],
                            start=(k == 0),
                            stop=(k == _KC - 1),
                        )
                    ht = hpool.tile(
                        [128, nl], f32r, tag=f"h_{ci}_{m}", name=f"h_{ci}_{m}"
                    )
                    nc.scalar.activation(
                        ht[:, :], ps[:, :], act.Relu, bias=b_sb[:, m : m + 1]
                    )
                    h_sb[(ci, m)] = ht
                    if ci == 0 and m == 0:
                        # Flow control: block the (in-order) gpsimd queue
                        # until chunk0 is underway, then release x2+/W2 —
                        # they must not compete with the startup loads.
                        nc.gpsimd.tensor_copy(scratch[:, :], ht[:, 0:1])
                        for cj in range(2, len(widths)):
                            nc.gpsimd.dma_start(
                                out=xb_sb[cj][:, :, :],
                                in_=xd[
                                    :,
                                    _KC * offsets[cj] : _KC
                                    * (offsets[cj] + widths[cj]),
                                ],
                            )
                        nc.gpsimd.dma_start(
                            out=w2b_sb[:, :, 0:256], in_=w2d[:, :, 0:256]
                        )
                        nc.gpsimd.dma_start(
                            out=w2b_sb[:, :, 256:_D], in_=w2d[:, :, 256:_D]
                        )
                        # Remaining upcasts, in deadline order.
                        for cj in range(2, len(widths)):
                            nc.vector.tensor_copy(
                                x_sb[cj][:, :, :], xb_sb[cj][:, :, :]
                            )
                        nc.vector.tensor_copy(
                            w2_sb[:, :, 0:256], w2b_sb[:, :, 0:256]
                        )
                        nc.vector.tensor_copy(
                            w2_sb[:, :, 256:_D], w2b_sb[:, :, 256:_D]
                        )

            # Phase 2: L2 (y = W2.T h + b2), chunks in reverse order so
            # the kernel tail drains the small lead-in chunk.
            for ci in reversed(range(len(widths))):
                nl, n0 = widths[ci], offsets[ci]
                yt = ypool.tile([128, md, nl], bf16, tag="y", name=f"y_{ci}")
                for d in range(md):
                    ps = psp.tile([128, nl], fp32, tag="ps", name=f"ps2_{ci}_{d}")
                    for m in range(mh):
                        nc.tensor.matmul(
                            ps[:, :],
                            w2_sb[:, m, d * 128 : (d + 1) * 128],
                            h_sb[(ci, m)][:, :],
                            start=(m == 0),
                            stop=(m == mh - 1),
                        )
                    nc.vector.tensor_scalar_add(
                        yt[:, d, :], ps[:, :], b_sb[:, mh + d : mh + d + 1]
                    )
                    if ci == 0 and d == md - 2:
                        # Final chunk: flush d0..d2 early so the kernel
                        # tail is a single small d-group store.
                        nc.sync.dma_start(
                            out=od[:, 0 : md - 1, n0 : n0 + nl],
                            in_=yt[:, 0 : md - 1, :],
                        )
                if ci == 0:
                    nc.sync.dma_start(
                        out=od[:, md - 1 :, n0 : n0 + nl], in_=yt[:, md - 1 :, :]
                    )
                else:
                    nc.sync.dma_start(out=od[:, :, n0 : n0 + nl], in_=yt[:, :, :])

    nc.compile()
    return nc


def _get_bass(C: int):
    nc = _kernel_cache.get(C)
    if nc is None:
        nc = _build_bass(C)
        _kernel_cache[C] = nc
    return nc


def _prepare_in_maps(latents, actions, policy_indices, W1, b1, W2, b2):
    """Expert-parallel dispatch: returns (in_maps, C, order, offs, counts)."""
    latents = np.asarray(latents, dtype=np.float32)
    actions = np.asarray(actions, dtype=np.float32)
    pi = np.asarray(policy_indices).astype(np.int64)
    W1 = np.asarray(W1, dtype=np.float32)
    b1 = np.asarray(b1, dtype=np.float32)
    W2 = np.asarray(W2, dtype=np.float32)
    b2 = np.asarray(b2, dtype=np.float32)

    B = latents.shape[0]
    counts = np.bincount(pi, minlength=_P)
    order = np.argsort(pi, kind="stable")
    offs = np.concatenate(([0], np.cumsum(counts)))

    # Exact capacity (rounded to 8 cols): matmul free size has no
    # 128-alignment requirement, and every padded column costs PE passes.
    C = max(256, int(math.ceil(counts.max() / 8)) * 8)

    x = np.empty((B, _DA), dtype=np.float32)
    x[:, :_D] = latents
    x[:, _D:] = actions
    x_sorted = x[order]

    mh, md = _MH, _MD
    widths = _chunks(C)
    noff = [sum(widths[:i]) for i in range(len(widths))]
    in_maps = []
    for p in range(_P):
        xp = np.zeros((C, _KC * 128), dtype=np.float32)
        xp[: counts[p], :_DA] = x_sorted[offs[p] : offs[p + 1]]
        # k4 rows 64..127 duplicate rows 0..63 (pairs with the odd
        # m-blocks' k4 weights relocated to partitions 64..127).
        xp[:, _DA : _DA + 64] = xp[:, _D:_DA]
        xr = xp.T.reshape(_KC, 128, C).transpose(1, 0, 2).astype(ml_dtypes.bfloat16)
        # chunk-major: chunk ci is one contiguous segment per partition.
        xq = np.empty((128, _KC * C), dtype=ml_dtypes.bfloat16)
        for ci, nl in enumerate(widths):
            n0 = noff[ci]
            xq[:, _KC * n0 : _KC * (n0 + nl)] = xr[:, :, n0 : n0 + nl].reshape(
                128, _KC * nl
            )
        w1p = np.zeros((_KC * 128, _H), dtype=np.float32)
        w1p[:_DA] = W1[p]
        # m-block-major: [128, mh, KC*128]; odd m-blocks' k4 weights
        # relocated to partitions 64..127 (zeros elsewhere), matching
        # the duplicated x k4 rows.
        w1m = (
            w1p.reshape(_KC, 128, mh, 128)
            .transpose(1, 2, 0, 3)
            .reshape(128, mh, _KC, 128)
            .copy()
        )
        w1m[64:128, 1::2, 4, :] = w1m[0:64, 1::2, 4, :]
        w1m[0:64, 1::2, 4, :] = 0.0
        w1q = np.ascontiguousarray(w1m.reshape(128, mh, _KC * 128)).astype(
            ml_dtypes.bfloat16
        )
        w2q = (
            np.ascontiguousarray(W2[p].reshape(mh, 128, _D).transpose(1, 0, 2))
            .astype(ml_dtypes.bfloat16)
        )
        bq = np.empty((128, mh + md), dtype=np.float32)
        bq[:, :mh] = b1[p].reshape(mh, 128).T
        bq[:, mh:] = b2[p].reshape(md, 128).T
        in_maps.append({"xq": xq, "w1q": w1q, "w2q": w2q, "bq": bq})
    return in_maps, C, order, offs, counts


def kernel(latents, actions, policy_indices, W1, b1, W2, b2):
    from concourse.bass_utils import run_bass_kernel_spmd

    in_maps, C, order, offs, counts = _prepare_in_maps(
        latents, actions, policy_indices, W1, b1, W2, b2
    )
    nc = _get_bass(C)
    results = run_bass_kernel_spmd(nc, in_maps, list(range(_N_CORES))).results

    B = np.asarray(latents).shape[0]
    out = np.empty((B, _D), dtype=np.float32)
    for p in range(_P):
        oq = np.asarray(results[p]["oq"])  # [128, 4, C] bf16
        yT = oq.transpose(1, 0, 2).reshape(_D, C)
        out[order[offs[p] : offs[p + 1]]] = yT[:, : counts[p]].T.astype(np.float32)
    return out


# revision 7
# speedup vs baseline: 1.0143x; 1.0143x over previous
"""MoE routed dynamics kernel for Trainium2 (8 NeuronCores, expert-parallel).

Problem: for each row b of a [B, D+A] input, route through one of P=8
two-layer MLPs selected by policy_indices[b]:
    h = relu(x @ W1[p] + b1[p]);  y = h @ W2[p] + b2[p]

Sharding: expert-parallel. Core p owns expert p's weights and processes
the rows routed to expert p (host-side gather by expert, padded to a
common capacity C; inverse scatter at unshard).

Design (v5; all constants from trace analysis on this silicon):
- Matmul dtype is float32r (N=512 pitch ~231ns vs bf16 ~259ns; fp32r
  needs N>=256 for 1 cycle/col), EXCEPT chunk0's L1 which runs bf16
  directly: inputs are already bf16-rounded in DRAM so bf16 MACs are
  numerically identical, and it removes the x0 upcast from the startup
  critical path (first-transfer DMA latency is ~3.7us after doorbell,
  so the first real matmul lands ~11.3us; warmups ramp the HAM clock
  gate meanwhile).
- Per-DGE-ring DMA bandwidth is far below the ~358GB/s bus, so the load
  stream is spread across three rings in consumption order, with W1
  m-blocks alternating scalar/gpsimd so their arrival cadence beats
  chunk0's ~0.6us/block consumption. x2+ and W2 are released by a
  gpsimd read gated on chunk0's first relu (in-order-queue flow
  control) so they never compete with startup-critical loads.
- Everything stages bf16 (half bus bytes): x, W1, W2. All upcasts run
  on DVE only (gpsimd CAST is ~5x slower, and concurrent DVE+gpsimd
  casts throttle each other ~19x), emitted in cast-deadline order.
- W1 is m-block-major in DRAM; odd m-blocks' k4 weights are relocated
  to partitions 64..127 with x's k4 rows 64..127 duplicating rows
  0..63 (layout also supports a packed row-tiled k4, which measured
  net-negative due to the sub-row LDWEIGHTS drain bubble).
- L1 keeps all h tiles resident in SBUF; L2 runs chunks in reverse so
  the tail drains the small lead-in chunk, whose d0..d2 groups are
  flushed early; output stored bf16, upcast on host.
"""

import math

import numpy as np
import ml_dtypes

_B = 16384
_P = 8
_D = 512
_A = 64
_H = 1024
_DA = _D + _A   # 576
_KC = 5         # K chunks over DA padded to 5*128=640
_N_CORES = 8
_MH = _H // 128  # 8 L1 output groups
_MD = _D // 128  # 4 L2 output groups

_WARMUP_N = 256
_WARMUPS = 15

_kernel_cache: dict = {}


def _chunks(C: int):
    """Column chunking: 256-wide lead-in (small first x transfer, so the
    PE starts sooner), then 512-wide steady chunks and a >=256 tail
    (fp32r needs N>=256 for full rate)."""
    assert C >= 256, C
    if C <= 512:
        return [C]
    out = [256]
    rem = C - 256
    while rem >= 1024:
        out.append(512)
        rem -= 512
    if rem <= 512:
        out.append(rem)
    elif rem - 512 >= 256:
        out += [512, rem - 512]
    else:
        out += [rem - 256, 256]
    return out


def _build_bass(C: int):
    import concourse.bacc as bacc
    import concourse.mybir as mybir
    from concourse.tile import TileContext

    fp32 = mybir.dt.float32
    f32r = mybir.dt.float32r
    bf16 = mybir.dt.bfloat16
    act = mybir.ActivationFunctionType

    widths = _chunks(C)
    offsets = [sum(widths[:i]) for i in range(len(widths))]
    mh, md = _MH, _MD

    nc = bacc.Bacc()
    xd = nc.declare_dram_parameter("xq", [128, _KC * C], bf16, isOutput=False)
    w1d = nc.declare_dram_parameter("w1q", [128, mh, _KC * 128], bf16, isOutput=False)
    w2d = nc.declare_dram_parameter("w2q", [128, mh, _D], bf16, isOutput=False)
    bd = nc.declare_dram_parameter("bq", [128, mh + md], fp32, isOutput=False)
    od = nc.declare_dram_parameter("oq", [128, md, C], bf16, isOutput=True)

    with TileContext(nc) as tc:
        with (
            tc.tile_pool(name="wpool", bufs=1) as wpool,
            tc.tile_pool(name="xpool", bufs=1) as xpool,
            tc.tile_pool(name="hpool", bufs=1) as hpool,
            tc.tile_pool(name="ypool", bufs=2) as ypool,
            tc.tile_pool(name="psum", bufs=8, space="PSUM") as psp,
        ):
            w1b_sb = wpool.tile([128, mh, _KC, 128], bf16, tag="w1b")
            w1_sb = wpool.tile([128, mh, _KC, 128], f32r, tag="w1")
            w2b_sb = wpool.tile([128, mh, _D], bf16, tag="w2b")
            w2_sb = wpool.tile([128, mh, _D], f32r, tag="w2")
            b_sb = wpool.tile([128, mh + md], fp32, tag="b")
            warm_sb = wpool.tile([128, 128 + _WARMUP_N], bf16, tag="warm")
            scratch = wpool.tile([128, 1], f32r, tag="scratch")

            xb_sb = [
                xpool.tile([128, _KC, nl], bf16, tag=f"xb{ci}", name=f"xb{ci}")
                for ci, nl in enumerate(widths)
            ]
            x_sb = [None, None] + [
                xpool.tile([128, _KC, nl], f32r, tag=f"x{ci}", name=f"x{ci}")
                for ci, nl in enumerate(widths[2:], start=2)
            ]

            nc.vector.memset(warm_sb[:, :], 0.0)

            # Per-transfer DMA latency is ~2-2.5us once several rings
            # compete, so W1 ships as four 2-m-block transfers (328KB)
            # alternating scalar/gpsimd — cadence ~2 blocks / 1.4us vs
            # chunk0's ~0.6us/block consumption. x0 (startup-critical),
            # biases, and x1 ride sync.
            nc.sync.dma_start(out=xb_sb[0][:, :, :], in_=xd[:, 0 : _KC * widths[0]])
            nc.sync.dma_start(
                out=xb_sb[1][:, :, :],
                in_=xd[:, _KC * offsets[1] : _KC * (offsets[1] + widths[1])],
            )
            nc.scalar.dma_start(out=w1b_sb[:, 0:2, :, :], in_=w1d[:, 0:2, :])
            nc.gpsimd.dma_start(out=w1b_sb[:, 2:4, :, :], in_=w1d[:, 2:4, :])
            nc.scalar.dma_start(out=b_sb[:, :], in_=bd[:, :])
            nc.scalar.dma_start(out=w1b_sb[:, 4:6, :, :], in_=w1d[:, 4:6, :])
            nc.gpsimd.dma_start(out=w1b_sb[:, 6:8, :, :], in_=w1d[:, 6:8, :])

            # All upcasts on DVE (deadline order; the f32r W1 blocks are
            # first needed by chunk2 since chunks 0-1 run bf16-direct).
            for m in range(mh):
                nc.vector.tensor_copy(w1_sb[:, m, :, :], w1b_sb[:, m, :, :])

            # PE warmup: the clock gate holds the PE at reduced rate for
            # ~3.4us of sustained activity; burn the x0 DMA wait.
            for _ in range(_WARMUPS):
                wp = psp.tile([128, _WARMUP_N], fp32, tag="ps", name="warmps")
                nc.tensor.matmul(
                    wp[:, :], warm_sb[:, 0:128], warm_sb[:, 128 : 128 + _WARMUP_N],
                    start=True, stop=True,
                )

            # Phase 1: L1 (h = relu(W1.T x + b1)). chunk0 consumes the
            # bf16 tiles directly; later chunks use the f32r upcasts.
            # All h tiles stay resident in SBUF.
            h_sb: dict = {}
            for ci, nl in enumerate(widths):
                if ci <= 1:
                    w_of = lambda m, k: w1b_sb[:, m, k, :]
                    xc = xb_sb[ci]
                else:
                    w_of = lambda m, k: w1_sb[:, m, k, :]
                    xc = x_sb[ci]
                for m in range(mh):
                    ps = psp.tile([128, nl], fp32, tag="ps", name=f"ps1_{ci}_{m}")
                    for k in range(_KC):
                        nc.tensor.matmul(
                            ps[:, :],
                            w_of(m, k),
                            xc[:, k, :],
                            start=(k == 0),
                            stop=(k == _KC - 1),
                        )
                    ht = hpool.tile(
                        [128, nl], f32r, tag=f"h_{ci}_{m}", name=f"h_{ci}_{m}"
                    )
                    nc.scalar.activation(
                        ht[:, :], ps[:, :], act.Relu, bias=b_sb[:, m : m + 1]
                    )
                    h_sb[(ci, m)] = ht
                    if ci == 0 and m == 0:
                        # Flow control: block the (in-order) gpsimd queue
                        # until chunk0 is underway, then release x2+/W2 —
                        # they must not compete with the startup loads.
                        nc.gpsimd.tensor_copy(scratch[:, :], ht[:, 0:1])
                        for cj in range(2, len(widths)):
                            nc.gpsimd.dma_start(
                                out=xb_sb[cj][:, :, :],
                                in_=xd[
                                    :,
                                    _KC * offsets[cj] : _KC
                                    * (offsets[cj] + widths[cj]),
                                ],
                            )
                        nc.gpsimd.dma_start(
                            out=w2b_sb[:, :, 0:256], in_=w2d[:, :, 0:256]
                        )
                        nc.gpsimd.dma_start(
                            out=w2b_sb[:, :, 256:_D], in_=w2d[:, :, 256:_D]
                        )
                        # Remaining upcasts, in deadline order.
                        for cj in range(2, len(widths)):
                            nc.vector.tensor_copy(
                                x_sb[cj][:, :, :], xb_sb[cj][:, :, :]
                            )
                        nc.vector.tensor_copy(
                            w2_sb[:, :, 0:256], w2b_sb[:, :, 0:256]
                        )
                        nc.vector.tensor_copy(
                            w2_sb[:, :, 256:_D], w2b_sb[:, :, 256:_D]
                        )

            # Phase 2: L2 (y = W2.T h + b2), chunks in reverse order so
            # the kernel tail drains the small lead-in chunk.
            for ci in reversed(range(len(widths))):
                nl, n0 = widths[ci], offsets[ci]
                yt = ypool.tile([128, md, nl], bf16, tag="y", name=f"y_{ci}")
                for d in range(md):
                    ps = psp.tile([128, nl], fp32, tag="ps", name=f"ps2_{ci}_{d}")
                    for m in range(mh):
                        nc.tensor.matmul(
                            ps[:, :],
                            w2_sb[:, m, d * 128 : (d + 1) * 128],
                            h_sb[(ci, m)][:, :],
                            start=(m == 0),
                            stop=(m == mh - 1),
                        )
                    nc.vector.tensor_scalar_add(
                        yt[:, d, :], ps[:, :], b_sb[:, mh + d : mh + d + 1]
                    )
                    if ci == 0 and d == md - 2:
                        # Final chunk: flush d0..d2 early so the kernel
                        # tail is a single small d-group store.
                        nc.sync.dma_start(
                            out=od[:, 0 : md - 1, n0 : n0 + nl],
                            in_=yt[:, 0 : md - 1, :],
                        )
                if ci == 0:
                    nc.sync.dma_start(
                        out=od[:, md - 1 :, n0 : n0 + nl], in_=yt[:, md - 1 :, :]
                    )
                else:
                    nc.sync.dma_start(out=od[:, :, n0 : n0 + nl], in_=yt[:, :, :])

    nc.compile()
    return nc


def _get_bass(C: int):
    nc = _kernel_cache.get(C)
    if nc is None:
        nc = _build_bass(C)
        _kernel_cache[C] = nc
    return nc


def _prepare_in_maps(latents, actions, policy_indices, W1, b1, W2, b2):
    """Expert-parallel dispatch: returns (in_maps, C, order, offs, counts)."""
    latents = np.asarray(latents, dtype=np.float32)
    actions = np.asarray(actions, dtype=np.float32)
    pi = np.asarray(policy_indices).astype(np.int64)
    W1 = np.asarray(W1, dtype=np.float32)
    b1 = np.asarray(b1, dtype=np.float32)
    W2 = np.asarray(W2, dtype=np.float32)
    b2 = np.asarray(b2, dtype=np.float32)

    B = latents.shape[0]
    counts = np.bincount(pi, minlength=_P)
    order = np.argsort(pi, kind="stable")
    offs = np.concatenate(([0], np.cumsum(counts)))

    # Exact capacity (rounded to 8 cols): matmul free size has no
    # 128-alignment requirement, and every padded column costs PE passes.
    C = max(256, int(math.ceil(counts.max() / 8)) * 8)

    x = np.empty((B, _DA), dtype=np.float32)
    x[:, :_D] = latents
    x[:, _D:] = actions
    x_sorted = x[order]

    mh, md = _MH, _MD
    widths = _chunks(C)
    noff = [sum(widths[:i]) for i in range(len(widths))]
    in_maps = []
    for p in range(_P):
        xp = np.zeros((C, _KC * 128), dtype=np.float32)
        xp[: counts[p], :_DA] = x_sorted[offs[p] : offs[p + 1]]
        # k4 rows 64..127 duplicate rows 0..63 (pairs with the odd
        # m-blocks' k4 weights relocated to partitions 64..127).
        xp[:, _DA : _DA + 64] = xp[:, _D:_DA]
        xr = xp.T.reshape(_KC, 128, C).transpose(1, 0, 2).astype(ml_dtypes.bfloat16)
        # chunk-major: chunk ci is one contiguous segment per partition.
        xq = np.empty((128, _KC * C), dtype=ml_dtypes.bfloat16)
        for ci, nl in enumerate(widths):
            n0 = noff[ci]
            xq[:, _KC * n0 : _KC * (n0 + nl)] = xr[:, :, n0 : n0 + nl].reshape(
                128, _KC * nl
            )
        w1p = np.zeros((_KC * 128, _H), dtype=np.float32)
        w1p[:_DA] = W1[p]
        # m-block-major: [128, mh, KC*128]; odd m-blocks' k4 weights
        # relocated to partitions 64..127 (zeros elsewhere), matching
        # the duplicated x k4 rows.
        w1m = (
            w1p.reshape(_KC, 128, mh, 128)
            .transpose(1, 2, 0, 3)
            .reshape(128, mh, _KC, 128)
            .copy()
        )
        w1m[64:128, 1::2, 4, :] = w1m[0:64, 1::2, 4, :]
        w1m[0:64, 1::2, 4, :] = 0.0
        w1q = np.ascontiguousarray(w1m.reshape(128, mh, _KC * 128)).astype(
            ml_dtypes.bfloat16
        )
        w2q = (
            np.ascontiguousarray(W2[p].reshape(mh, 128, _D).transpose(1, 0, 2))
            .astype(ml_dtypes.bfloat16)
        )
        bq = np.empty((128, mh + md), dtype=np.float32)
        bq[:, :mh] = b1[p].reshape(mh, 128).T
        bq[:, mh:] = b2[p].reshape(md, 128).T
        in_maps.append({"xq": xq, "w1q": w1q, "w2q": w2q, "bq": bq})
    return in_maps, C, order, offs, counts


def kernel(latents, actions, policy_indices, W1, b1, W2, b2):
    from concourse.bass_utils import run_bass_kernel_spmd

    in_maps, C, order, offs, counts = _prepare_in_maps(
        latents, actions, policy_indices, W1, b1, W2, b2
    )
    nc = _get_bass(C)
    results = run_bass_kernel_spmd(nc, in_maps, list(range(_N_CORES))).results

    B = np.asarray(latents).shape[0]
    out = np.empty((B, _D), dtype=np.float32)
    for p in range(_P):
        oq = np.asarray(results[p]["oq"])  # [128, 4, C] bf16
        yT = oq.transpose(1, 0, 2).reshape(_D, C)
        out[order[offs[p] : offs[p + 1]]] = yT[:, : counts[p]].T.astype(np.float32)
    return out


# revision 9
# speedup vs baseline: 1.0760x; 1.0609x over previous
"""MoE routed dynamics kernel for Trainium2 (8 NeuronCores, expert-parallel).

Problem: for each row b of a [B, D+A] input, route through one of P=8
two-layer MLPs selected by policy_indices[b]:
    h = relu(x @ W1[p] + b1[p]);  y = h @ W2[p] + b2[p]

Sharding: expert-parallel. Core p owns expert p's weights and processes
the rows routed to expert p (host-side gather by expert, padded to a
common capacity C; inverse scatter at unshard).

Design (v5; all constants from trace analysis on this silicon):
- Matmul dtype is float32r (N=512 pitch ~231ns vs bf16 ~259ns; fp32r
  needs N>=256 for 1 cycle/col), EXCEPT chunk0's L1 which runs bf16
  directly: inputs are already bf16-rounded in DRAM so bf16 MACs are
  numerically identical, and it removes the x0 upcast from the startup
  critical path (first-transfer DMA latency is ~3.7us after doorbell,
  so the first real matmul lands ~11.3us; warmups ramp the HAM clock
  gate meanwhile).
- Per-DGE-ring DMA bandwidth is far below the ~358GB/s bus, so the load
  stream is spread across three rings in consumption order, with W1
  m-blocks alternating scalar/gpsimd so their arrival cadence beats
  chunk0's ~0.6us/block consumption. x2+ and W2 are released by a
  gpsimd read gated on chunk0's first relu (in-order-queue flow
  control) so they never compete with startup-critical loads.
- Everything stages bf16 (half bus bytes): x, W1, W2. All upcasts run
  on DVE only (gpsimd CAST is ~5x slower, and concurrent DVE+gpsimd
  casts throttle each other ~19x), emitted in cast-deadline order.
- W1 is m-block-major in DRAM; odd m-blocks' k4 weights are relocated
  to partitions 64..127 with x's k4 rows 64..127 duplicating rows
  0..63 (layout also supports a packed row-tiled k4, which measured
  net-negative due to the sub-row LDWEIGHTS drain bubble).
- L1 keeps all h tiles resident in SBUF; L2 runs chunks in reverse so
  the tail drains the small lead-in chunk, whose d0..d2 groups are
  flushed early; output stored bf16, upcast on host.
"""

import math

import numpy as np
import ml_dtypes

_B = 16384
_P = 8
_D = 512
_A = 64
_H = 1024
_DA = _D + _A   # 576
_KC = 5         # K chunks over DA padded to 5*128=640
_N_CORES = 8
_MH = _H // 128  # 8 L1 output groups
_MD = _D // 128  # 4 L2 output groups

_WARMUP_N = 256
_WARMUPS = 15

_kernel_cache: dict = {}


def _chunks(C: int):
    """Column chunking: 256-wide lead-in (small first x transfer, so the
    PE starts sooner), then 512-wide steady chunks and a >=256 tail
    (fp32r needs N>=256 for full rate)."""
    assert C >= 256, C
    if C <= 512:
        return [C]
    out = [256]
    rem = C - 256
    while rem >= 1024:
        out.append(512)
        rem -= 512
    if rem <= 512:
        out.append(rem)
    elif rem - 512 >= 256:
        out += [512, rem - 512]
    else:
        out += [rem - 256, 256]
    return out


def _build_bass(C: int, biases_zero: bool):
    import concourse.bacc as bacc
    import concourse.mybir as mybir
    from concourse.tile import TileContext

    fp32 = mybir.dt.float32
    f32r = mybir.dt.float32r
    bf16 = mybir.dt.bfloat16
    act = mybir.ActivationFunctionType

    widths = _chunks(C)
    offsets = [sum(widths[:i]) for i in range(len(widths))]
    mh, md = _MH, _MD

    nc = bacc.Bacc()
    xd = nc.declare_dram_parameter("xq", [128, _KC * C], bf16, isOutput=False)
    w1d = nc.declare_dram_parameter("w1q", [128, mh, _KC * 128], bf16, isOutput=False)
    w2d = nc.declare_dram_parameter("w2q", [128, mh, _D], bf16, isOutput=False)
    bd = nc.declare_dram_parameter("bq", [128, mh + md], fp32, isOutput=False)
    od = nc.declare_dram_parameter("oq", [128, md, C], bf16, isOutput=True)

    with TileContext(nc) as tc:
        with (
            tc.tile_pool(name="wpool", bufs=1) as wpool,
            tc.tile_pool(name="xpool", bufs=1) as xpool,
            tc.tile_pool(name="hpool", bufs=1) as hpool,
            tc.tile_pool(name="ypool", bufs=2) as ypool,
            tc.tile_pool(name="psum", bufs=8, space="PSUM") as psp,
        ):
            w1b_sb = wpool.tile([128, mh, _KC, 128], bf16, tag="w1b")
            w1_sb = wpool.tile([128, mh, _KC, 128], f32r, tag="w1")
            w2b_sb = wpool.tile([128, mh, _D], bf16, tag="w2b")
            w2_sb = wpool.tile([128, mh, _D], f32r, tag="w2")
            b_sb = wpool.tile([128, mh + md], fp32, tag="b")
            warm_sb = wpool.tile([128, 128 + _WARMUP_N], bf16, tag="warm")
            scratch = wpool.tile([128, 1], f32r, tag="scratch")

            xb_sb = [
                xpool.tile([128, _KC, nl], bf16, tag=f"xb{ci}", name=f"xb{ci}")
                for ci, nl in enumerate(widths)
            ]
            x_sb = [None, None] + [
                xpool.tile([128, _KC, nl], f32r, tag=f"x{ci}", name=f"x{ci}")
                for ci, nl in enumerate(widths[2:], start=2)
            ]

            nc.vector.memset(warm_sb[:, :], 0.0)

            # Per-transfer DMA latency is ~2-2.5us once several rings
            # compete, so W1 ships as four 2-m-block transfers (328KB)
            # alternating scalar/gpsimd — cadence ~2 blocks / 1.4us vs
            # chunk0's ~0.6us/block consumption. x0 (startup-critical),
            # biases, and x1 ride sync.
            nc.sync.dma_start(out=xb_sb[0][:, :, :], in_=xd[:, 0 : _KC * widths[0]])
            nc.sync.dma_start(
                out=xb_sb[1][:, :, :],
                in_=xd[:, _KC * offsets[1] : _KC * (offsets[1] + widths[1])],
            )
            nc.scalar.dma_start(out=w1b_sb[:, 0:2, :, :], in_=w1d[:, 0:2, :])
            nc.gpsimd.dma_start(out=w1b_sb[:, 2:4, :, :], in_=w1d[:, 2:4, :])
            nc.scalar.dma_start(out=w1b_sb[:, 4:6, :, :], in_=w1d[:, 4:6, :])
            nc.gpsimd.dma_start(out=w1b_sb[:, 6:8, :, :], in_=w1d[:, 6:8, :])
            if biases_zero:
                # Biases are zero in this workload: a memset avoids a DMA
                # transfer (each costs ~2.5us of ring latency at startup).
                nc.vector.memset(b_sb[:, :], 0.0)
            else:
                nc.sync.dma_start(out=b_sb[:, :], in_=bd[:, :])

            # All upcasts on DVE (deadline order; the f32r W1 blocks are
            # first needed by chunk2 since chunks 0-1 run bf16-direct).
            for m in range(mh):
                nc.vector.tensor_copy(w1_sb[:, m, :, :], w1b_sb[:, m, :, :])

            # PE warmup: the clock gate holds the PE at reduced rate for
            # ~3.4us of sustained activity; burn the x0 DMA wait.
            for _ in range(_WARMUPS):
                wp = psp.tile([128, _WARMUP_N], fp32, tag="ps", name="warmps")
                nc.tensor.matmul(
                    wp[:, :], warm_sb[:, 0:128], warm_sb[:, 128 : 128 + _WARMUP_N],
                    start=True, stop=True,
                )

            # Phase 1: L1 (h = relu(W1.T x + b1)). chunk0 consumes the
            # bf16 tiles directly; later chunks use the f32r upcasts.
            # All h tiles stay resident in SBUF.
            h_sb: dict = {}
            for ci, nl in enumerate(widths):
                if ci <= 1:
                    w_of = lambda m, k: w1b_sb[:, m, k, :]
                    xc = xb_sb[ci]
                else:
                    w_of = lambda m, k: w1_sb[:, m, k, :]
                    xc = x_sb[ci]
                for m in range(mh):
                    ps = psp.tile([128, nl], fp32, tag="ps", name=f"ps1_{ci}_{m}")
                    for k in range(_KC):
                        nc.tensor.matmul(
                            ps[:, :],
                            w_of(m, k),
                            xc[:, k, :],
                            start=(k == 0),
                            stop=(k == _KC - 1),
                        )
                    ht = hpool.tile(
                        [128, nl], f32r, tag=f"h_{ci}_{m}", name=f"h_{ci}_{m}"
                    )
                    nc.scalar.activation(
                        ht[:, :], ps[:, :], act.Relu, bias=b_sb[:, m : m + 1]
                    )
                    h_sb[(ci, m)] = ht
                    if ci == 0 and m == 0:
                        # Flow control: block the (in-order) gpsimd queue
                        # until chunk0 is underway, then release x2+/W2 —
                        # they must not compete with the startup loads.
                        nc.gpsimd.tensor_copy(scratch[:, :], ht[:, 0:1])
                        for cj in range(2, len(widths)):
                            nc.gpsimd.dma_start(
                                out=xb_sb[cj][:, :, :],
                                in_=xd[
                                    :,
                                    _KC * offsets[cj] : _KC
                                    * (offsets[cj] + widths[cj]),
                                ],
                            )
                        nc.gpsimd.dma_start(
                            out=w2b_sb[:, :, 0:256], in_=w2d[:, :, 0:256]
                        )
                        nc.gpsimd.dma_start(
                            out=w2b_sb[:, :, 256:_D], in_=w2d[:, :, 256:_D]
                        )
                        # Remaining upcasts, in deadline order.
                        for cj in range(2, len(widths)):
                            nc.vector.tensor_copy(
                                x_sb[cj][:, :, :], xb_sb[cj][:, :, :]
                            )
                        nc.vector.tensor_copy(
                            w2_sb[:, :, 0:256], w2b_sb[:, :, 0:256]
                        )
                        nc.vector.tensor_copy(
                            w2_sb[:, :, 256:_D], w2b_sb[:, :, 256:_D]
                        )

            # Phase 2: L2 (y = W2.T h + b2), chunks in reverse order so
            # the kernel tail drains the small lead-in chunk.
            for ci in reversed(range(len(widths))):
                nl, n0 = widths[ci], offsets[ci]
                yt = ypool.tile([128, md, nl], bf16, tag="y", name=f"y_{ci}")
                for d in range(md):
                    ps = psp.tile([128, nl], fp32, tag="ps", name=f"ps2_{ci}_{d}")
                    for m in range(mh):
                        nc.tensor.matmul(
                            ps[:, :],
                            w2_sb[:, m, d * 128 : (d + 1) * 128],
                            h_sb[(ci, m)][:, :],
                            start=(m == 0),
                            stop=(m == mh - 1),
                        )
                    nc.vector.tensor_scalar_add(
                        yt[:, d, :], ps[:, :], b_sb[:, mh + d : mh + d + 1]
                    )
                    if ci == 0 and d == md - 2:
                        # Final chunk: flush d0..d2 early so the kernel
                        # tail is a single small d-group store.
                        nc.sync.dma_start(
                            out=od[:, 0 : md - 1, n0 : n0 + nl],
                            in_=yt[:, 0 : md - 1, :],
                        )
                if ci == 0:
                    nc.sync.dma_start(
                        out=od[:, md - 1 :, n0 : n0 + nl], in_=yt[:, md - 1 :, :]
                    )
                else:
                    nc.sync.dma_start(out=od[:, :, n0 : n0 + nl], in_=yt[:, :, :])

    nc.compile()
    return nc


def _get_bass(C: int, biases_zero: bool = True):
    key = (C, biases_zero)
    nc = _kernel_cache.get(key)
    if nc is None:
        nc = _build_bass(C, biases_zero)
        _kernel_cache[key] = nc
    return nc


def _prepare_in_maps(latents, actions, policy_indices, W1, b1, W2, b2):
    """Expert-parallel dispatch: returns (in_maps, C, order, offs, counts)."""
    latents = np.asarray(latents, dtype=np.float32)
    actions = np.asarray(actions, dtype=np.float32)
    pi = np.asarray(policy_indices).astype(np.int64)
    W1 = np.asarray(W1, dtype=np.float32)
    b1 = np.asarray(b1, dtype=np.float32)
    W2 = np.asarray(W2, dtype=np.float32)
    b2 = np.asarray(b2, dtype=np.float32)

    B = latents.shape[0]
    counts = np.bincount(pi, minlength=_P)
    order = np.argsort(pi, kind="stable")
    offs = np.concatenate(([0], np.cumsum(counts)))

    # Exact capacity (rounded to 8 cols): matmul free size has no
    # 128-alignment requirement, and every padded column costs PE passes.
    C = max(256, int(math.ceil(counts.max() / 8)) * 8)

    x = np.empty((B, _DA), dtype=np.float32)
    x[:, :_D] = latents
    x[:, _D:] = actions
    x_sorted = x[order]

    mh, md = _MH, _MD
    widths = _chunks(C)
    noff = [sum(widths[:i]) for i in range(len(widths))]
    in_maps = []
    for p in range(_P):
        xp = np.zeros((C, _KC * 128), dtype=np.float32)
        xp[: counts[p], :_DA] = x_sorted[offs[p] : offs[p + 1]]
        # k4 rows 64..127 duplicate rows 0..63 (pairs with the odd
        # m-blocks' k4 weights relocated to partitions 64..127).
        xp[:, _DA : _DA + 64] = xp[:, _D:_DA]
        xr = xp.T.reshape(_KC, 128, C).transpose(1, 0, 2).astype(ml_dtypes.bfloat16)
        # chunk-major: chunk ci is one contiguous segment per partition.
        xq = np.empty((128, _KC * C), dtype=ml_dtypes.bfloat16)
        for ci, nl in enumerate(widths):
            n0 = noff[ci]
            xq[:, _KC * n0 : _KC * (n0 + nl)] = xr[:, :, n0 : n0 + nl].reshape(
                128, _KC * nl
            )
        w1p = np.zeros((_KC * 128, _H), dtype=np.float32)
        w1p[:_DA] = W1[p]
        # m-block-major: [128, mh, KC*128]; odd m-blocks' k4 weights
        # relocated to partitions 64..127 (zeros elsewhere), matching
        # the duplicated x k4 rows.
        w1m = (
            w1p.reshape(_KC, 128, mh, 128)
            .transpose(1, 2, 0, 3)
            .reshape(128, mh, _KC, 128)
            .copy()
        )
        w1m[64:128, 1::2, 4, :] = w1m[0:64, 1::2, 4, :]
        w1m[0:64, 1::2, 4, :] = 0.0
        w1q = np.ascontiguousarray(w1m.reshape(128, mh, _KC * 128)).astype(
            ml_dtypes.bfloat16
        )
        w2q = (
            np.ascontiguousarray(W2[p].reshape(mh, 128, _D).transpose(1, 0, 2))
            .astype(ml_dtypes.bfloat16)
        )
        bq = np.empty((128, mh + md), dtype=np.float32)
        bq[:, :mh] = b1[p].reshape(mh, 128).T
        bq[:, mh:] = b2[p].reshape(md, 128).T
        in_maps.append({"xq": xq, "w1q": w1q, "w2q": w2q, "bq": bq})
    return in_maps, C, order, offs, counts


def kernel(latents, actions, policy_indices, W1, b1, W2, b2):
    from concourse.bass_utils import run_bass_kernel_spmd

    in_maps, C, order, offs, counts = _prepare_in_maps(
        latents, actions, policy_indices, W1, b1, W2, b2
    )
    bz = not (np.any(np.asarray(b1)) or np.any(np.asarray(b2)))
    nc = _get_bass(C, bz)
    results = run_bass_kernel_spmd(nc, in_maps, list(range(_N_CORES))).results

    B = np.asarray(latents).shape[0]
    out = np.empty((B, _D), dtype=np.float32)
    for p in range(_P):
        oq = np.asarray(results[p]["oq"])  # [128, 4, C] bf16
        yT = oq.transpose(1, 0, 2).reshape(_D, C)
        out[order[offs[p] : offs[p + 1]]] = yT[:, : counts[p]].T.astype(np.float32)
    return out
